# revision 1
# baseline (speedup 1.0000x reference)
import numpy as np

# nn_AXRFeatureLoss: hardcoded problem shapes
B, C, H, W = 8, 256, 96, 96
Cq, K = 32, 6
CA_W, CC_W = 0.0005, 1e-05
EPS = 1e-6


def _channel_stats(x):
    # per-channel mean/std over (N,H,W); unbiased std (ddof=1), like torch.std
    xd = x.astype(np.float64)
    mean = xd.mean(axis=(0, 2, 3))
    std = np.sqrt(xd.var(axis=(0, 2, 3), ddof=1))
    return mean.astype(np.float32), std.astype(np.float32)


def _norm_host(x):
    mean, std = _channel_stats(x)
    return (x - mean[None, :, None, None]) / (std[None, :, None, None] + EPS)


def _numpy_per_image(sn, tn, w_cls, wq, bq, wk, bk, wv, bv, gamma1):
    # sn, tn: (C,H,W) normalized single image; returns (ca_sq, cc_sq) partial sums
    def softmax(m, axis):
        m = m - m.max(axis=axis, keepdims=True)
        e = np.exp(m)
        return e / e.sum(axis=axis, keepdims=True)

    def causal(x):
        M = np.einsum('chw,oc->ohw', x, w_cls).reshape(K, H * W)
        sm = softmax(M, 1)
        return np.einsum('kp,cp->kc', sm, x.reshape(C, H * W))

    def ccnet(x):
        q = np.einsum('chw,oc->ohw', x, wq) + bq[:, None, None]
        k = np.einsum('chw,oc->ohw', x, wk) + bk[:, None, None]
        v = np.einsum('chw,oc->ohw', x, wv) + bv[:, None, None]
        eH = np.einsum('ciw,cjw->iwj', q, k)
        i_idx = np.arange(H)
        eH[i_idx[:, None], :, i_idx[:, None]] = -np.inf
        eW = np.einsum('chi,chj->hij', q, k)
        att = softmax(np.concatenate([eH, eW], axis=2), 2)
        attH, attW = att[..., :H], att[..., H:]
        outH = np.einsum('cjw,iwj->ciw', v, attH)
        outW = np.einsum('chj,hij->chi', v, attW)
        return gamma1 * (outH + outW) + x

    ca_sq = float(np.sum((causal(tn) - causal(sn)) ** 2))
    cc_sq = float(np.sum((ccnet(tn) - ccnet(sn)) ** 2))
    return ca_sq, cc_sq


def _run_numpy(sn, tn, w_cls, wq, bq, wk, bk, wv, bv, gamma1):
    ca_tot, cc_tot = 0.0, 0.0
    for n in range(B):
        ca, cc = _numpy_per_image(sn[n], tn[n], w_cls, wq, bq, wk, bk, wv, bv,
                                  float(gamma1[0]))
        ca_tot += ca
        cc_tot += cc
    return ca_tot, cc_tot


def _run_jax_pmap(sn, tn, w_cls, wq, bq, wk, bk, wv, bv, gamma1):
    # Data-parallel over B: one image per NeuronCore, partial loss sums
    # reduced on host.
    import jax
    import jax.numpy as jnp

    devs = jax.devices()
    assert len(devs) >= B

    eye = np.eye(H, dtype=bool)[:, None, :]

    def per_image(sn1, tn1, w_cls, wq, bq, wk, bk, wv, bv, gamma1):
        def conv(x, w, b=None):
            y = jnp.einsum('chw,oc->ohw', x, w)
            return y if b is None else y + b[:, None, None]

        def causal(x):
            M = conv(x, w_cls).reshape(K, H * W)
            sm = jax.nn.softmax(M, axis=1)
            return jnp.einsum('kp,cp->kc', sm, x.reshape(C, H * W))

        def ccnet(x):
            q = conv(x, wq, bq)
            k = conv(x, wk, bk)
            v = conv(x, wv, bv)
            eH = jnp.einsum('ciw,cjw->iwj', q, k)
            eH = jnp.where(eye, -jnp.inf, eH)
            eW = jnp.einsum('chi,chj->hij', q, k)
            att = jax.nn.softmax(jnp.concatenate([eH, eW], axis=2), axis=2)
            attH, attW = att[..., :H], att[..., H:]
            outH = jnp.einsum('cjw,iwj->ciw', v, attH)
            outW = jnp.einsum('chj,hij->chi', v, attW)
            return gamma1[0] * (outH + outW) + x

        ca_sq = jnp.sum((causal(tn1) - causal(sn1)) ** 2)
        cc_sq = jnp.sum((ccnet(tn1) - ccnet(sn1)) ** 2)
        return ca_sq, cc_sq

    f = jax.pmap(per_image,
                 in_axes=(0, 0, None, None, None, None, None, None, None, None),
                 devices=devs[:B])
    ca_sq, cc_sq = f(sn, tn, w_cls, wq, bq, wk, bk, wv, bv, gamma1)
    return float(np.sum(np.asarray(ca_sq))), float(np.sum(np.asarray(cc_sq)))


def kernel(**inputs):
    preds_S = np.asarray(inputs['preds_S'], dtype=np.float32)
    preds_T = np.asarray(inputs['preds_T'], dtype=np.float32)
    w_cls = np.asarray(inputs['w_cls'], dtype=np.float32)
    wq = np.asarray(inputs['wq'], dtype=np.float32)
    bq = np.asarray(inputs['bq'], dtype=np.float32)
    wk = np.asarray(inputs['wk'], dtype=np.float32)
    bk = np.asarray(inputs['bk'], dtype=np.float32)
    wv = np.asarray(inputs['wv'], dtype=np.float32)
    bv = np.asarray(inputs['bv'], dtype=np.float32)
    gamma1 = np.asarray(inputs['gamma1'], dtype=np.float32)

    sn = _norm_host(preds_S)
    tn = _norm_host(preds_T)

    try:
        ca_tot, cc_tot = _run_jax_pmap(sn, tn, w_cls, wq, bq, wk, bk, wv, bv,
                                       gamma1)
    except Exception:
        ca_tot, cc_tot = _run_numpy(sn, tn, w_cls, wq, bq, wk, bk, wv, bv,
                                    gamma1)

    loss = (ca_tot / B) * CA_W + (cc_tot / B) * CC_W
    return np.array(loss, dtype=np.float32)


# revision 4
# speedup vs baseline: 1.3351x; 1.3351x over previous
import numpy as np

# nn_AXRFeatureLoss: hardcoded problem shapes
B, C, H, W = 8, 256, 96, 96
Cq, K = 32, 6
CA_W, CC_W = 0.0005, 1e-05
EPS = 1e-6


def _channel_stats(x):
    # per-channel mean/std over (N,H,W); unbiased std (ddof=1), like torch.std
    xd = x.astype(np.float64)
    mean = xd.mean(axis=(0, 2, 3))
    std = np.sqrt(xd.var(axis=(0, 2, 3), ddof=1))
    return mean.astype(np.float32), std.astype(np.float32)


def _norm_host(x):
    mean, std = _channel_stats(x)
    return (x - mean[None, :, None, None]) / (std[None, :, None, None] + EPS)


def _numpy_per_image(sn, tn, w_cls, wq, bq, wk, bk, wv, bv, gamma1):
    # sn, tn: (C,H,W) normalized single image; returns (ca_sq, cc_sq) partial sums
    def softmax(m, axis):
        m = m - m.max(axis=axis, keepdims=True)
        e = np.exp(m)
        return e / e.sum(axis=axis, keepdims=True)

    def causal(x):
        M = np.einsum('chw,oc->ohw', x, w_cls).reshape(K, H * W)
        sm = softmax(M, 1)
        return np.einsum('kp,cp->kc', sm, x.reshape(C, H * W))

    def ccnet(x):
        q = np.einsum('chw,oc->ohw', x, wq) + bq[:, None, None]
        k = np.einsum('chw,oc->ohw', x, wk) + bk[:, None, None]
        v = np.einsum('chw,oc->ohw', x, wv) + bv[:, None, None]
        eH = np.einsum('ciw,cjw->iwj', q, k)
        i_idx = np.arange(H)
        eH[i_idx[:, None], :, i_idx[:, None]] = -np.inf
        eW = np.einsum('chi,chj->hij', q, k)
        att = softmax(np.concatenate([eH, eW], axis=2), 2)
        attH, attW = att[..., :H], att[..., H:]
        outH = np.einsum('cjw,iwj->ciw', v, attH)
        outW = np.einsum('chj,hij->chi', v, attW)
        return gamma1 * (outH + outW) + x

    ca_sq = float(np.sum((causal(tn) - causal(sn)) ** 2))
    cc_sq = float(np.sum((ccnet(tn) - ccnet(sn)) ** 2))
    return ca_sq, cc_sq


def _run_numpy(sn, tn, w_cls, wq, bq, wk, bk, wv, bv, gamma1):
    ca_tot, cc_tot = 0.0, 0.0
    for n in range(B):
        ca, cc = _numpy_per_image(sn[n], tn[n], w_cls, wq, bq, wk, bk, wv, bv,
                                  float(gamma1[0]))
        ca_tot += ca
        cc_tot += cc
    return ca_tot, cc_tot


_PMAP_CACHE = {}


def _get_pmap_fn():
    if 'f' in _PMAP_CACHE:
        return _PMAP_CACHE['f']
    import jax
    import jax.numpy as jnp

    devs = jax.devices()
    assert len(devs) >= B

    eye = np.eye(H, dtype=bool)[:, None, :]

    def per_image(s1, t1, stat_s, stat_t, w_cls, wq, bq, wk, bk, wv, bv,
                  gamma1):
        # s1/t1: (C,H,W) raw image; stats: (2,C) mean/std — normalize on device
        sn1 = (s1 - stat_s[0][:, None, None]) / (stat_s[1][:, None, None] + EPS)
        tn1 = (t1 - stat_t[0][:, None, None]) / (stat_t[1][:, None, None] + EPS)
        def conv(x, w, b=None):
            y = jnp.einsum('chw,oc->ohw', x, w)
            return y if b is None else y + b[:, None, None]

        def causal(x):
            M = conv(x, w_cls).reshape(K, H * W)
            sm = jax.nn.softmax(M, axis=1)
            return jnp.einsum('kp,cp->kc', sm, x.reshape(C, H * W))

        def ccnet(x):
            q = conv(x, wq, bq)
            k = conv(x, wk, bk)
            v = conv(x, wv, bv)
            eH = jnp.einsum('ciw,cjw->iwj', q, k)
            eH = jnp.where(eye, -jnp.inf, eH)
            eW = jnp.einsum('chi,chj->hij', q, k)
            att = jax.nn.softmax(jnp.concatenate([eH, eW], axis=2), axis=2)
            attH, attW = att[..., :H], att[..., H:]
            outH = jnp.einsum('cjw,iwj->ciw', v, attH)
            outW = jnp.einsum('chj,hij->chi', v, attW)
            return gamma1[0] * (outH + outW) + x

        ca_sq = jnp.sum((causal(tn1) - causal(sn1)) ** 2)
        cc_sq = jnp.sum((ccnet(tn1) - ccnet(sn1)) ** 2)
        return ca_sq, cc_sq

    f = jax.pmap(per_image,
                 in_axes=(0, 0) + (None,) * 10,
                 devices=devs[:B])
    _PMAP_CACHE['f'] = f
    return f


def _run_jax_pmap(preds_S, preds_T, stat_s, stat_t, w_cls, wq, bq, wk, bk, wv,
                  bv, gamma1):
    f = _get_pmap_fn()
    ca_sq, cc_sq = f(preds_S, preds_T, stat_s, stat_t, w_cls, wq, bq, wk, bk,
                     wv, bv, gamma1)
    return float(np.sum(np.asarray(ca_sq))), float(np.sum(np.asarray(cc_sq)))


def kernel(**inputs):
    preds_S = np.asarray(inputs['preds_S'], dtype=np.float32)
    preds_T = np.asarray(inputs['preds_T'], dtype=np.float32)
    w_cls = np.asarray(inputs['w_cls'], dtype=np.float32)
    wq = np.asarray(inputs['wq'], dtype=np.float32)
    bq = np.asarray(inputs['bq'], dtype=np.float32)
    wk = np.asarray(inputs['wk'], dtype=np.float32)
    bk = np.asarray(inputs['bk'], dtype=np.float32)
    wv = np.asarray(inputs['wv'], dtype=np.float32)
    bv = np.asarray(inputs['bv'], dtype=np.float32)
    gamma1 = np.asarray(inputs['gamma1'], dtype=np.float32)

    mean_s, std_s = _channel_stats(preds_S)
    mean_t, std_t = _channel_stats(preds_T)
    stat_s = np.stack([mean_s, std_s])
    stat_t = np.stack([mean_t, std_t])

    try:
        ca_tot, cc_tot = _run_jax_pmap(preds_S, preds_T, stat_s, stat_t,
                                       w_cls, wq, bq, wk, bk, wv, bv, gamma1)
    except Exception:
        sn = (preds_S - mean_s[None, :, None, None]) / (std_s[None, :, None, None] + EPS)
        tn = (preds_T - mean_t[None, :, None, None]) / (std_t[None, :, None, None] + EPS)
        ca_tot, cc_tot = _run_numpy(sn, tn, w_cls, wq, bq, wk, bk, wv, bv,
                                    gamma1)

    loss = (ca_tot / B) * CA_W + (cc_tot / B) * CC_W
    return np.array(loss, dtype=np.float32)
